# revision 13
# baseline (speedup 1.0000x reference)
"""Trainium2 Bass kernel for CombinedLossExp72 — v4 (norm-sorted group premax).

Per 128-token tile (8 cores x 2048 tokens each, data-parallel over batch):
  PE:   G = x @ cb_sorted^T            (32 bf16 matmuls into [P,4,64,8] PSUM)
  DVE:  Hx = group-max over 8 sorted-by-norm codes, straight out of PSUM
        (one tensor_reduce per GEMM half; codebook rows pre-sorted by ||c||
        so each group shares c2/2 and 1/||c|| to ~1e-4)
  Pool: Hp = Hx + sh3 (group c2h row + per-token positive-group spike)
        sE = Hx * rc_g
  DVE:  exact top-16 groups on Hp: max8 -> match_replace -> max8 -> match_replace
  ACT:  E = exp(sE / (T*||x||))
  DVE:  negsum = sum (Hp==SENT) * E     (fused select+accum)
  feature/triplet/pos-logit ride along on ACT/Pool/DVE.
Inputs are packed host-side: one [P,4,D] f32 DMA (x,t,tn,pos), one [P,4,128]
bf16 DMA (transposed x chunks), one [P,512] f32 DMA (sh3) per tile.
"""

import numpy as np
import ml_dtypes
from contextlib import ExitStack

B, T, D, K = 8, 2048, 512, 4096
NCORES = 8
TOK = (B * T) // NCORES      # tokens per core
P = 128
NTILES = TOK // P            # 16
NCHUNK = D // P              # 4 contraction chunks
GRP = 16                     # premax group size
KG = K // GRP                # 256 groups
MARGIN, TEMP = 0.2, 0.1
FEATURE_W, TRIPLET_W, CONTRASTIVE_W = 1.0, 1.0, 0.5
# bf16-exact sentinels (powers of two survive the bf16 round-trip, so the
# f32 immediates in match_replace / is_equal compare equal to stored values)
SPIKE = -float(2.0 ** 99)    # added at the positive code's group
SENT = -float(2.0 ** 100)    # match_replace sentinel marking selected groups


def emit(tc, ins, outs, ntiles=NTILES, reps=1):
    import concourse.bass as bass  # noqa: F401
    from concourse import mybir

    nc = tc.nc
    f32 = mybir.dt.float32
    bf16 = mybir.dt.bfloat16
    AF = mybir.ActivationFunctionType
    OP = mybir.AluOpType
    AX = mybir.AxisListType

    fp8 = mybir.dt.float8e4
    big_nat = ins["big_nat"]     # [TOK, 4, D] f32: x, t, tn, pos
    xTt = ins["xTt"]             # [NTILES, P, NCHUNK, P] fp8e4
    cb_in = ins["cb_in"]         # [P, NCHUNK, K] fp8e4 (norm-sorted, chunked)
    rcg_in = ins["rcg_in"]       # [P, KG] f32 (replicated group inv-norms)
    sh3_in = ins["sh3"]          # [TOK, KG] f32 (c2h_g row + group spike)
    out_part = outs["out_part"]

    with ExitStack() as ctx:
        const = ctx.enter_context(tc.tile_pool(name="const", bufs=1))
        iop = ctx.enter_context(tc.tile_pool(name="io", bufs=3))
        work = ctx.enter_context(tc.tile_pool(name="work", bufs=2))
        sm = ctx.enter_context(tc.tile_pool(name="sm", bufs=6))
        colsp = ctx.enter_context(tc.tile_pool(name="cols", bufs=1))
        scr = ctx.enter_context(tc.tile_pool(name="scr", bufs=2))
        psum = ctx.enter_context(tc.tile_pool(name="psum", bufs=2, space="PSUM"))

        # ---- constants (loaded once, on the gpsimd DMA queue) ----
        cb8 = const.tile([P, NCHUNK, K], fp8, name="cb8")
        nc.gpsimd.dma_start(cb8[:], cb_in[:])
        rcg_sb = const.tile([P, 2, 4, KG // 8], f32, name="rcg_sb")
        nc.gpsimd.dma_start(rcg_sb[:], rcg_in[:])
        margin_sb = const.tile([P, 1], f32, name="margin_sb")
        nc.vector.memset(margin_sb[:], MARGIN)
        nlt_sb = const.tile([P, 1], f32, name="nlt_sb")
        nc.vector.memset(nlt_sb[:], -float(np.log(TEMP)))

        featcols = colsp.tile([P, ntiles], f32, name="featcols")
        tripcols = colsp.tile([P, ntiles], f32, name="tripcols")
        cecols = colsp.tile([P, ntiles], f32, name="cecols")
        # per-tile scalars land in columns; the whole ce tail runs once,
        # batched over all tiles, after the loop
        posdcols = colsp.tile([P, ntiles], f32, name="posdcols")
        p2cols = colsp.tile([P, ntiles], f32, name="p2cols")
        ndcols = colsp.tile([P, ntiles], f32, name="ndcols")
        rxcols = colsp.tile([P, ntiles], f32, name="rxcols")
        rpcols = colsp.tile([P, ntiles], f32, name="rpcols")
        negcols = colsp.tile([P, ntiles], f32, name="negcols")

        pending_sel = None   # (t, Hx3, sh3_t, rxoT) awaiting selection

        def _select(p):
            """Top-16 selection + masked negsum for a tile whose GEMM+premax
            already ran — emitted one tile late so the next tile's PSUM
            reduces sit at the head of the DVE queue, not behind this chain."""
            st, Hx3, sh3_t = p
            Hp = work.tile([P, 2, 4, KG // 8], bf16, tag="Hp")
            nc.gpsimd.tensor_tensor(Hp[:], Hx3[:], sh3_t[:], OP.add)
            m1 = sm.tile([P, 8], bf16, tag="m1")
            nc.vector.max(m1[:], Hp[:])
            nc.vector.match_replace(Hp[:], m1[:], Hp[:], SENT)
            m2 = sm.tile([P, 8], bf16, tag="m2")
            nc.vector.max(m2[:], Hp[:])
            nc.vector.match_replace(Hp[:], m2[:], Hp[:], SENT)
            sE = work.tile([P, 2, 4, KG // 8], f32, tag="sE")
            nc.gpsimd.tensor_tensor(sE[:], Hx3[:], rcg_sb[:], OP.mult)
            E = work.tile([P, 2, 4, KG // 8], bf16, tag="E")
            nc.scalar.activation(E[:], sE[:], AF.Exp,
                                 scale=rxcols[:, st:st + 1])
            Ez = work.tile([P, 2, 4, KG // 8], bf16, tag="Ez")
            nc.vector.scalar_tensor_tensor(Ez[:], Hp[:], SENT, E[:],
                                           OP.is_equal, OP.mult,
                                           accum_out=negcols[:, st:st + 1])

        for t in [tt for _ in range(reps) for tt in range(ntiles)]:
            rs = slice(t * P, (t + 1) * P)
            big_t = iop.tile([P, 4, D], f32, tag="big_t")
            nc.sync.dma_start(big_t[:], big_nat[rs, :, :])
            x_t, t_t, tn_t, p_t = (big_t[:, i, :] for i in range(4))
            xT_t = iop.tile([P, NCHUNK, P], fp8, tag="xT_t")
            nc.sync.dma_start(xT_t[:], xTt[t, :, :, :])
            sh3_t = iop.tile([P, 2, 4, KG // 8], f32, tag="sh3_t")
            nc.sync.dma_start(sh3_t[:], sh3_in[rs, :])

            # ---- per-token norms (sqrt-free: stay in the ln/exp ACT set) ----
            s0 = scr.tile([P, D], f32, tag="scr512")
            x2 = sm.tile([P, 1], f32, tag="x2")
            nc.scalar.activation(s0[:], x_t, AF.Square, accum_out=x2[:])
            lx2 = sm.tile([P, 1], f32, tag="lx2")
            nc.scalar.activation(lx2[:], x2[:], AF.Ln)
            nc.scalar.activation(rxcols[:, t:t + 1], lx2[:], AF.Exp,
                                 scale=-0.5, bias=nlt_sb[:])  # 1/(T*||x||)

            s1 = scr.tile([P, D], f32, tag="scr512")
            nc.scalar.activation(s1[:], p_t, AF.Square,
                                 accum_out=p2cols[:, t:t + 1])

            # ---- feature + triplet (Pool subtracts + ACT squares) ----
            dsc = scr.tile([P, D], f32, tag="dsc")
            nc.gpsimd.tensor_tensor(dsc[:], x_t, t_t, OP.subtract)
            s2 = scr.tile([P, D], f32, tag="scr512")
            nc.scalar.activation(s2[:], dsc[:], AF.Square,
                                 accum_out=featcols[:, t:t + 1])

            nsc = scr.tile([P, D], f32, tag="dsc")
            nc.gpsimd.tensor_tensor(nsc[:], x_t, tn_t, OP.subtract)
            s3 = scr.tile([P, D], f32, tag="scr512")
            nc.scalar.activation(s3[:], nsc[:], AF.Square,
                                 accum_out=ndcols[:, t:t + 1])

            # ---- positive logit dot (Pool mult + ACT copy-accum row sum) ----
            s4 = scr.tile([P, D], f32, tag="dsc")
            nc.gpsimd.tensor_tensor(s4[:], x_t, p_t, OP.mult)
            s5 = scr.tile([P, D], f32, tag="scr512")
            nc.scalar.activation(s5[:], s4[:], AF.Copy,
                                 accum_out=posdcols[:, t:t + 1])

            # ---- fp8 DoubleRow GEMM halves + group-max out of PSUM (DVE) ----
            Hx3 = work.tile([P, 2, 4, KG // 8], f32, tag="Hx3")
            for h in range(2):
                pg = psum.tile([P, 4, KG // 8, GRP], f32, tag="psum",
                               name=f"pg{h}")
                for kk in range(0, NCHUNK, 2):
                    for j in range(4):
                        cs = slice(h * 2048 + j * 512, h * 2048 + (j + 1) * 512)
                        nc.tensor.matmul(pg[:, j], xT_t[:, kk:kk + 2, :],
                                         cb8[:, kk:kk + 2, cs],
                                         perf_mode=mybir.MatmulPerfMode.DoubleRow,
                                         start=(kk == 0),
                                         stop=(kk == NCHUNK - 2))
                nc.vector.tensor_reduce(Hx3[:, h], pg[:], AX.X, OP.max)

            # ---- deferred selection of the previous tile, so the reduces
            # above stay at the head of the DVE queue ----
            if pending_sel is not None:
                _select(pending_sel)
            pending_sel = (t, Hx3, sh3_t)

        _select(pending_sel)

        # ---- batched norm/triplet tails over all tiles ----
        lpc = colsp.tile([P, ntiles], f32, name="lpc")
        nc.scalar.activation(lpc[:], p2cols[:], AF.Ln)
        nc.scalar.activation(rpcols[:], lpc[:], AF.Exp, scale=-0.5)
        ldc = colsp.tile([P, ntiles], f32, name="ldc")
        nc.scalar.activation(ldc[:], featcols[:], AF.Ln)
        pdc = colsp.tile([P, ntiles], f32, name="pdc")
        nc.scalar.activation(pdc[:], ldc[:], AF.Exp, scale=0.5)
        lnc = colsp.tile([P, ntiles], f32, name="lnc")
        nc.scalar.activation(lnc[:], ndcols[:], AF.Ln)
        ndc = colsp.tile([P, ntiles], f32, name="ndc")
        nc.scalar.activation(ndc[:], lnc[:], AF.Exp, scale=0.5)
        tvc = colsp.tile([P, ntiles], f32, name="tvc")
        nc.vector.tensor_tensor(tvc[:], pdc[:], ndc[:], OP.subtract)
        nc.scalar.activation(tripcols[:], tvc[:], AF.Relu, bias=margin_sb[:])

        # ---- ce tail, batched over all tiles in [P, ntiles] ops ----
        q1 = colsp.tile([P, ntiles], f32, name="q1")
        nc.vector.tensor_tensor(q1[:], posdcols[:], rxcols[:], OP.mult)
        l0c = colsp.tile([P, ntiles], f32, name="l0c")
        nc.vector.tensor_tensor(l0c[:], q1[:], rpcols[:], OP.mult)
        pec = colsp.tile([P, ntiles], f32, name="pec")
        nc.scalar.activation(pec[:], l0c[:], AF.Exp)
        uc = colsp.tile([P, ntiles], f32, name="uc")
        nc.vector.tensor_tensor(uc[:], negcols[:], pec[:], OP.add)
        lsec = colsp.tile([P, ntiles], f32, name="lsec")
        nc.scalar.activation(lsec[:], uc[:], AF.Ln)
        nc.vector.tensor_tensor(cecols[:], lsec[:], l0c[:], OP.subtract)

        outsb = colsp.tile([P, 4], f32, name="outsb")
        nc.vector.memset(outsb[:, 3:4], 0.0)
        nc.vector.tensor_reduce(outsb[:, 0:1], featcols[:], AX.X, OP.add)
        nc.vector.tensor_reduce(outsb[:, 1:2], tripcols[:], AX.X, OP.add)
        nc.vector.tensor_reduce(outsb[:, 2:3], cecols[:], AX.X, OP.add)
        nc.sync.dma_start(out_part[:], outsb[:])


def _patch_act_tables():
    """Bias the act-table-load placement pass toward the one set
    (natural_log_exp_and_others) that contains every func this kernel uses
    (square/ln/exp/relu), so the whole program needs a single table load."""
    import concourse.bacc as bacc_mod
    if getattr(bacc_mod, "_act_tables_patched", False):
        return
    orig = bacc_mod.get_activation_tables
    target = "natural_log_exp_and_others"

    def patched(module_arch):
        tabs = orig(module_arch)
        full = tabs[target]
        return {name: (s if name == target else s - full)
                for name, s in tabs.items()}

    bacc_mod.get_activation_tables = patched
    bacc_mod._act_tables_patched = True


def build(ntiles=NTILES, reps=1):
    """Build + compile the Bacc program. Returns nc."""
    import concourse.bacc as bacc
    import concourse.tile as tile
    from concourse import mybir

    _patch_act_tables()

    f32 = mybir.dt.float32
    fp8 = mybir.dt.float8e4

    nc = bacc.Bacc("TRN2", target_bir_lowering=False, debug=False,
                   enable_asserts=False, num_devices=NCORES)
    ins = {
        "big_nat": nc.dram_tensor("big_nat", [TOK, 4, D], f32, kind="ExternalInput").ap(),
        "xTt": nc.dram_tensor("xTt", [NTILES, P, NCHUNK, P], fp8, kind="ExternalInput").ap(),
        "cb_in": nc.dram_tensor("cb_in", [P, NCHUNK, K], fp8, kind="ExternalInput").ap(),
        "rcg_in": nc.dram_tensor("rcg_in", [P, KG], f32, kind="ExternalInput").ap(),
        "sh3": nc.dram_tensor("sh3", [TOK, KG], f32, kind="ExternalInput").ap(),
    }
    outs = {
        "out_part": nc.dram_tensor("out_part", [P, 4], f32, kind="ExternalOutput").ap(),
    }
    with tile.TileContext(nc) as tc:
        emit(tc, ins, outs, ntiles=ntiles, reps=reps)
    nc.compile()
    return nc


def make_in_maps(student_features, teacher_features, codebook, teacher_codes):
    """Host-side shard + layout prep. Returns list of 8 per-core input dicts."""
    x = np.ascontiguousarray(np.asarray(student_features, dtype=np.float32)).reshape(B * T, D)
    tch = np.ascontiguousarray(np.asarray(teacher_features, dtype=np.float32)).reshape(B, T, D)
    cb = np.ascontiguousarray(np.asarray(codebook, dtype=np.float32))
    codes = np.asarray(teacher_codes).reshape(B * T).astype(np.int64)

    c2 = (cb.astype(np.float64) ** 2).sum(axis=1)
    order = np.argsort(c2)
    cb_s = cb[order]
    c2_s = c2[order]
    inv_order = np.empty(K, dtype=np.int64)
    inv_order[order] = np.arange(K)
    codes_s = inv_order[codes]            # sorted-position of each token's code

    c2h_g = (-(c2_s / 2)).reshape(KG, GRP).mean(axis=1).astype(np.float32)
    rc_g = (1.0 / np.sqrt(c2_s)).reshape(KG, GRP).mean(axis=1).astype(np.float32)

    np8 = ml_dtypes.float8_e4m3
    # cb8[p, c, k] = cb_s[k, c*128+p]
    cb8 = np.ascontiguousarray(
        cb_s.T.reshape(NCHUNK, P, K).transpose(1, 0, 2)).astype(np8)
    rcg_rep = np.ascontiguousarray(np.broadcast_to(rc_g[None, :], (P, KG)))

    in_maps = []
    for b in range(NCORES):
        sl = slice(b * TOK, (b + 1) * TOK)
        xs = x[sl]
        codes_sl = codes[sl]
        big = np.empty((TOK, 4, D), dtype=np.float32)
        big[:, 0, :] = xs
        big[:, 1, :] = tch[b]
        big[:, 2, :] = tch[(b - 1) % B]
        big[:, 3, :] = cb[codes_sl]
        xT = np.ascontiguousarray(xs.T).astype(np8)  # [D, TOK]
        xTt = np.ascontiguousarray(
            xT.reshape(NCHUNK, P, NTILES, P).transpose(2, 1, 0, 3))
        sh3 = np.broadcast_to(c2h_g[None, :], (TOK, KG)).copy()
        sh3[np.arange(TOK), codes_s[sl] // GRP] = SPIKE
        in_maps.append({
            "big_nat": big,
            "xTt": xTt,
            "cb_in": cb8,
            "rcg_in": rcg_rep,
            "sh3": sh3,
        })
    return in_maps


def combine(results):
    """Combine per-core [128, 4] partials into the scalar loss."""
    feat = trip = ce = 0.0
    for r in results:
        p = np.asarray(r["out_part"], dtype=np.float64)
        feat += p[:, 0].sum()
        trip += p[:, 1].sum()
        ce += p[:, 2].sum()
    n = float(B * T)
    total = (FEATURE_W * feat / (n * D)
             + TRIPLET_W * trip / n
             + CONTRASTIVE_W * ce / n)
    return np.float32(total)


_NC_CACHE = None


def kernel(student_features, teacher_features, codebook, teacher_codes):
    global _NC_CACHE
    from concourse import bass_utils

    if _NC_CACHE is None:
        _NC_CACHE = build()
    nc = _NC_CACHE
    in_maps = make_in_maps(student_features, teacher_features, codebook,
                           teacher_codes)
    res = bass_utils.run_bass_kernel_spmd(nc, in_maps,
                                          core_ids=list(range(NCORES)))
    return combine(res.results)
